# revision 1
# baseline (speedup 1.0000x reference)
"""Multi-head causal attention (B=2, S=4096, D=512, H=8, DK=64) on 8 TRN2
NeuronCores.

Sharding: batch x head-pair. Core c handles batch c//4, heads {2*(c%4),
2*(c%4)+1} end-to-end through attention; head mixing for the output
projection happens via an AllGather of transposed per-head outputs (oT)
within each batch's 4-core group, after which each core applies the full
Wo to its 1024-row sequence slice.

Per-core dataflow (everything "T" is d-major, i.e. feature dim on SBUF
partitions, which is what the PE matmul contraction needs):
  QT/KT via PE transpose (fp32)  -> qT/kT = W^T @ XT      (f32r matmuls)
  VT via PE transpose (bf16)     -> v    = V @ Wv          (bf16, t-major)
  sT[t, sq] = k^T q (transposed scores; K=64 contraction)  (f32r)
  wT = exp(sT/8) via ScalarE straight out of PSUM (causal mask added on
       the 128x128 diagonal blocks only; strictly-above-diagonal blocks
       are never computed)
  oT_aug = [v | 1]^T @ wT accumulated over t-chunks in one PSUM bank;
       row 64 is the softmax denominator (no separate reduction pass)
  oT = oT_aug[:64] * (1/oT_aug[64]) broadcast via GPSIMD partition_broadcast
  oT (bf16) AllGathered within the 4-core batch group in three pieces
       (q-chunks 0-3 / 4-5 / 6-7), each fired as soon as its columns are
       done so only the last 0.25MB gather sits on the critical tail;
       out-proj rows are selected with partition_id-driven conditional
       DMAs + dynamic slices; y = oT_all^T @ Wo (bf16), stored fp32.

Engine budget (cost model, per core): PE ~181us (scores+oT accumulation
are the floor at 16.8M PSUM elements / 128 lanes each; fp32 input
transposes 41us), ScalarE ~153us (144 exp instructions over [128,1024]
PSUM groups), VectorE ~110us (PSUM evacuations), 3 collectives.
PSUM (8 banks): 2x alternating single-buffer score pools + 2 oT
accumulators + 2 double-buffered single-bank scratch tiles for the
transpose/projection/out-proj pipeline. oT columns are staged to the
DRAM bounce per chunk so each AllGather fires the moment its last chunk
normalizes; weight loads queue behind the first input slice.
The diagonal mask-add is one DVE op per group covering both heads via a
zero-stride (head-broadcast) mask AP. TimelineSim end-to-end: ~284us
(compute span ~218us; the rest is the cost model's pessimistic AllGather
floor plus conditional DMAs it cannot see are skipped).
"""

import sys

sys.path.insert(0, "/opt/trn_rl_repo")

import numpy as np

import concourse.bass as bass
import concourse.mybir as mybir
import concourse.tile as tile
from concourse import bacc
from concourse.bass import ds, ts
from concourse.bass_utils import run_bass_kernel_spmd
from concourse.masks import make_identity

B, S, D, H, DK = 2, 4096, 512, 8, 64
SQ, TC = 512, 128  # q-chunk rows, t-chunk rows
NSL = S // SQ  # 8 row slices
NCHUNK = S // TC  # 32 t-chunks
f32 = mybir.dt.float32
f32r = mybir.dt.float32r
bf16 = mybir.dt.bfloat16
AF = mybir.ActivationFunctionType
ALU = mybir.AluOpType

_CACHED_NC = None


def attention_chunk(nc, pool, sA, sB, otp, mask128, qt_sl, kt_sl, v_sl, ot_half,
                    bounce_in_part, crel, gi0):
    """Attention for q-chunk c, both heads, t-chunks 0..4(c+1)-1.

    Each scores/exp group holds one t-chunk for BOTH heads ([128, 2, 512]);
    oT accumulates per head in its own PSUM bank across the t loop."""
    c = len(qt_sl) - 1  # current q-chunk == latest slice
    n_tc = 4 * (c + 1)
    ot_ps = [None, None]
    for tcg in range(n_tc):
        r = tcg - 4 * c
        sl, lc = tcg // 4, tcg % 4
        n0 = 128 * r if r >= 0 else 0
        gi = gi0 + tcg
        sp = (sA if gi % 2 == 0 else sB).tile(
            [128, 2, 512], f32, tag="sA" if gi % 2 == 0 else "sB"
        )
        for h in range(2):
            nc.tensor.matmul(
                sp[:, h, n0:512],
                lhsT=kt_sl[sl][64 * h : 64 * h + 64, ts(lc, 128)],
                rhs=qt_sl[c][64 * h : 64 * h + 64, n0:512],
                start=True,
                stop=True,
            )
        if r >= 0:
            mask2 = bass.AP(
                tensor=mask128.tensor,
                offset=mask128.offset,
                ap=[mask128.ap[0], [0, 2], [1, 128]],
            )
            nc.vector.tensor_add(
                sp[:, :, n0 : n0 + 128], sp[:, :, n0 : n0 + 128], mask2
            )
        wt = pool("wt", 6).tile([128, 2, 512], bf16, tag="wt")
        nc.scalar.activation(wt, sp, AF.Exp, scale=0.125)
        for h in range(2):
            if tcg == 0:
                ot_ps[h] = otp.tile([128, 512], f32, tag="otp", name=f"otp_c{c}h{h}")
            nc.tensor.matmul(
                ot_ps[h][0:65, n0:512],
                lhsT=v_sl[sl][:, lc, 65 * h : 65 * h + 65],
                rhs=wt[:, h, n0:512],
                start=(tcg == 0),
                stop=(tcg == n_tc - 1),
            )
    for h in range(2):
        # one cheap copy releases the PSUM accumulator immediately; the
        # normalize chain then runs off the oT-accumulation critical path
        ot_raw = pool("otraw", 4).tile([65, 512], f32, tag="otraw",
                                       name=f"otraw_c{c}h{h}")
        nc.vector.tensor_copy(ot_raw, ot_ps[h][0:65, :])
        recip = pool("recip", 2).tile([1, 512], f32, tag="recip")
        nc.vector.reciprocal(recip, ot_raw[64:65, :])
        rbc = pool("rbc", 2).tile([64, 512], f32, tag="rbc")
        nc.gpsimd.partition_broadcast(rbc, recip)
        nc.vector.tensor_mul(
            ot_half[64 * h : 64 * h + 64, crel, :], ot_raw[0:64, :], rbc
        )
    nc.sync.dma_start(
        bounce_in_part[:, ts(crel, 512)], ot_half[:, crel, :]
    )


def _build_body(nc, tc, Q, K, V, Wq, Wk, Wv, Wo, Y):
    ctx_pools = {}

    def pool(name, bufs, space="SBUF"):
        if name not in ctx_pools:
            ctx_pools[name] = tc.alloc_tile_pool(name=name, bufs=bufs, space=space)
        return ctx_pools[name]

    def psum_pool(name, bufs):
        return pool(name, bufs, space="PSUM")

    const = pool("const", 1)
    ident32 = const.tile([128, 128], f32, tag="id32")
    make_identity(nc, ident32)
    ident16 = const.tile([128, 128], bf16, tag="id16")
    make_identity(nc, ident16)
    # additive causal mask for a 128x128 diagonal block: keep (0) where
    # col >= row, else -1e10
    mask128 = const.tile([128, 128], f32, tag="mask")
    nc.vector.memset(mask128, 0.0)
    nc.gpsimd.affine_select(
        out=mask128,
        in_=mask128,
        compare_op=ALU.is_ge,
        fill=-1e10,
        base=0,
        channel_multiplier=-1,
        pattern=[[1, 128]],
    )


    mm = psum_pool("mm", 2)  # [128, 512] single-bank tiles, double-buffered
    sA = psum_pool("sA", 1)  # [128, 2, 512] scores group (even)
    sB = psum_pool("sB", 1)  # [128, 2, 512] scores group (odd)
    otp = psum_pool("otp", 2)  # [128, 512] oT accumulator

    qt_sl, kt_sl, v_sl = [], [], []
    GI = [0]
    pid = nc.partition_id()
    hp = pid % 4
    ag_conds = [hp < 2, (hp > 1) & (hp < 3), hp > 2]
    off0 = (pid % 2) * 1024
    ot_all = [
        pool("otall", 4).tile([128, 1024], bf16, tag="otall", name=f"otall{dc}")
        for dc in range(4)
    ]

    # attention staging: heads paired per scores group (PE row-group
    # concurrency); oT written bf16, AllGathered in three overlapped pieces
    # (chunks 0-3 / 4-5 / 6-7) so only the last small AG sits on the tail
    AG_SPLIT = [(0, 4), (4, 6), (6, 8)]  # [c0, c1) chunk ranges
    ot_parts = [
        pool("ot", 1).tile([128, c1 - c0, 512], bf16, tag=f"otp{i}", name=f"otsb{i}")
        for i, (c0, c1) in enumerate(AG_SPLIT)
    ]
    dram = pool("dram", 1, space="DRAM")
    bounce_in = [
        dram.tile([128, (c1 - c0) * 512], bf16, tag=f"bin{i}", name=f"bin{i}")
        for i, (c0, c1) in enumerate(AG_SPLIT)
    ]
    bounce_out = [
        dram.tile([512, (c1 - c0) * 512], bf16, tag=f"bout{i}", name=f"bout{i}")
        for i, (c0, c1) in enumerate(AG_SPLIT)
    ]

    # ---------------- phase 1: load, transpose, project ----------------
    Qr = Q.rearrange("(s g p) d -> s p g d", p=128, g=4)
    Kr = K.rearrange("(s g p) d -> s p g d", p=128, g=4)
    Vr = V.rearrange("(s g p) d -> s p g d", p=128, g=4)

    wq_sb = wk_sb = wv_sb = wo_sb = None
    for s in range(NSL):
        # --- Q and K: fp32 path ---
        xq = pool("xin", 4).tile([128, 4, 512], f32, tag="xin")
        for g in range(4):
            nc.sync.dma_start(xq[:, g, :], Qr[s, :, g, :])
        xk = pool("xin", 4).tile([128, 4, 512], f32, tag="xin")
        for g in range(4):
            nc.sync.dma_start(xk[:, g, :], Kr[s, :, g, :])
        if s == 0:
            # weight loads queued behind the first slice so they don't
            # delay the first transposes
            wq_f = const.tile([128, 4, 128], f32, tag="wqf")
            nc.sync.dma_start(wq_f, Wq.rearrange("(c p) k -> p c k", p=128))
            wq_sb = const.tile([128, 4, 128], f32r, tag="wq")
            nc.vector.tensor_copy(wq_sb, wq_f)
            wk_f = const.tile([128, 4, 128], f32, tag="wkf")
            nc.sync.dma_start(wk_f, Wk.rearrange("(c p) k -> p c k", p=128))
            wk_sb = const.tile([128, 4, 128], f32r, tag="wk")
            nc.vector.tensor_copy(wk_sb, wk_f)
            wv_sb = const.tile([128, 4, 128], bf16, tag="wv")
            nc.gpsimd.dma_start(wv_sb, Wv.rearrange("(c p) k -> p c k", p=128))
            wo_sb = const.tile([128, 4, 512], bf16, tag="wo")
            nc.gpsimd.dma_start(wo_sb, Wo.rearrange("(c p) n -> p c n", p=128))
        xtq = pool("xt", 3).tile([128, 4, 512], f32r, tag="xtqk")
        xtk = pool("xt", 3).tile([128, 4, 512], f32r, tag="xtqk")
        for x_sl, xt_sb in ((xq, xtq), (xk, xtk)):
            for dc in range(4):
                t_ps = mm.tile([128, 512], f32, tag="mm", name=f"tps_{s}_{dc}")
                for g in range(4):
                    nc.tensor.transpose(
                        t_ps[:, ts(g, 128)], x_sl[:, g, ts(dc, 128)], ident32
                    )
                nc.vector.tensor_copy(xt_sb[:, dc, :], t_ps)
        # qT/kT projections (both heads of the pair): [128, 512]
        qt_ps = mm.tile([128, 512], f32, tag="mm")
        for dc in range(4):
            nc.tensor.matmul(
                qt_ps,
                lhsT=wq_sb[:, dc, :],
                rhs=xtq[:, dc, :],
                start=(dc == 0),
                stop=(dc == 3),
            )
        qt = pool("qt", NSL).tile([128, 512], f32r, tag="qt")
        nc.vector.tensor_copy(qt, qt_ps)
        qt_sl.append(qt)
        kt_ps = mm.tile([128, 512], f32, tag="mm")
        for dc in range(4):
            nc.tensor.matmul(
                kt_ps,
                lhsT=wk_sb[:, dc, :],
                rhs=xtk[:, dc, :],
                start=(dc == 0),
                stop=(dc == 3),
            )
        kt = pool("kt", NSL).tile([128, 512], f32r, tag="kt")
        nc.vector.tensor_copy(kt, kt_ps)
        kt_sl.append(kt)

        # --- V: bf16 path ---
        xv = pool("xinv", 2).tile([128, 4, 512], bf16, tag="xinv")
        nc.gpsimd.dma_start(xv, Vr[s])  # casting DMA f32 -> bf16
        xtv = pool("xtv", 3).tile([128, 4, 512], bf16, tag="xtv")
        for dc in range(4):
            t_ps = mm.tile([128, 512], bf16, tag="mm", name=f"tpsv_{s}_{dc}")
            for g in range(4):
                nc.tensor.transpose(
                    t_ps[:, ts(g, 128)], xv[:, g, ts(dc, 128)], ident16
                )
            nc.vector.tensor_copy(xtv[:, dc, :], t_ps)
        # v projection, t-major: per t-chunk [128, 2*64]; interleave into
        # v_aug [128, 4, 130] with a ones column per head at 65h+64
        vp = mm.tile([128, 512], f32, tag="mm")
        for tcl in range(4):
            for dc in range(4):
                nc.tensor.matmul(
                    vp[:, ts(tcl, 128)],
                    lhsT=xtv[:, dc, ts(tcl, 128)],
                    rhs=wv_sb[:, dc, :],
                    start=(dc == 0),
                    stop=(dc == 3),
                )
        va = pool("v", NSL).tile([128, 4, 130], bf16, tag="v")
        nc.vector.memset(va.rearrange("p c (h k) -> p c h k", k=65)[:, :, :, 64:65], 1.0)
        nc.vector.tensor_copy(
            va.rearrange("p c (h k) -> p c h k", k=65)[:, :, :, 0:64],
            vp.rearrange("p (c h k) -> p c h k", c=4, h=2),
        )
        v_sl.append(va)

        part = next(i for i, (c0, c1) in enumerate(AG_SPLIT) if c0 <= s < c1)
        attention_chunk(nc, pool, sA, sB, otp, mask128, qt_sl, kt_sl, v_sl,
                        ot_parts[part], bounce_in[part],
                        s - AG_SPLIT[part][0], GI[0])
        GI[0] += 4 * (s + 1)
        if s == AG_SPLIT[part][1] - 1:
            nc.gpsimd.collective_compute(
                "AllGather",
                ALU.bypass,
                replica_groups=[[0, 1, 2, 3], [4, 5, 6, 7]],
                ins=[bounce_in[part].opt()],
                outs=[bounce_out[part].opt()],
            )
            for dc in range(4):
                nc.sync.dma_start(
                    ot_all[dc],
                    bounce_out[part][ts(dc, 128), ds(off0, 1024)]
                    if part == 0
                    else bounce_out[part][ts(dc, 128), 0:1024],
                    cond=ag_conds[part],
                )

    # ------------- phase 3: gather my columns + output projection -------
    for st in range(8):
        ym = mm.tile([128, 512], f32, tag="mm", name=f"ym_{st}")
        for dc in range(4):
            nc.tensor.matmul(
                ym,
                lhsT=ot_all[dc][:, ts(st, 128)],
                rhs=wo_sb[:, dc, :],
                start=(dc == 0),
                stop=(dc == 3),
            )
        y_sb = pool("y", 4).tile([128, 512], f32, tag="y")
        if st % 2 == 0:
            nc.scalar.copy(y_sb, ym)
            nc.sync.dma_start(Y[ts(st, 128), :], y_sb)
        else:
            nc.vector.tensor_copy(y_sb, ym)
            nc.scalar.dma_start(Y[ts(st, 128), :], y_sb)

    for p in reversed(list(ctx_pools.values())):
        p.release()


def _build():
    global _CACHED_NC
    if _CACHED_NC is not None:
        return _CACHED_NC
    nc = bacc.Bacc("TRN2", num_devices=8)
    Q = nc.dram_tensor("Q", [S, D], f32, kind="ExternalInput")
    K = nc.dram_tensor("K", [S, D], f32, kind="ExternalInput")
    V = nc.dram_tensor("V", [S, D], f32, kind="ExternalInput")
    Wq = nc.dram_tensor("Wq", [D, 128], f32, kind="ExternalInput")
    Wk = nc.dram_tensor("Wk", [D, 128], f32, kind="ExternalInput")
    Wv = nc.dram_tensor("Wv", [D, 128], f32, kind="ExternalInput")
    Wo = nc.dram_tensor("Wo", [D, D], f32, kind="ExternalInput")
    Y = nc.dram_tensor("Y", [1024, D], f32, kind="ExternalOutput")
    with tile.TileContext(nc) as tcx:
        _build_body(nc, tcx, Q, K, V, Wq, Wk, Wv, Wo, Y)
    nc.finalize()
    _CACHED_NC = nc
    return nc


def _in_maps(inputs):
    Q, K, V = (np.asarray(inputs[k], np.float32) for k in ("Q", "K", "V"))
    Wq, Wk, Wv, Wo = (
        np.asarray(inputs[k], np.float32) for k in ("Wq", "Wk", "Wv", "Wo")
    )
    in_maps = []
    for c in range(8):
        b, hp = c // 4, c % 4
        in_maps.append(
            {
                "Q": np.ascontiguousarray(Q[b]),
                "K": np.ascontiguousarray(K[b]),
                "V": np.ascontiguousarray(V[b]),
                "Wq": np.ascontiguousarray(
                    np.concatenate([Wq[2 * hp], Wq[2 * hp + 1]], axis=1)
                ),
                "Wk": np.ascontiguousarray(
                    np.concatenate([Wk[2 * hp], Wk[2 * hp + 1]], axis=1)
                ),
                "Wv": np.ascontiguousarray(
                    np.concatenate([Wv[2 * hp], Wv[2 * hp + 1]], axis=1)
                ),
                "Wo": Wo,
            }
        )
    return in_maps


def kernel(Q, K, V, Wq, Wk, Wv, Wo):
    nc = _build()
    in_maps = _in_maps(
        {"Q": Q, "K": K, "V": V, "Wq": Wq, "Wk": Wk, "Wv": Wv, "Wo": Wo}
    )
    res = run_bass_kernel_spmd(nc, in_maps, core_ids=list(range(8)))
    out = np.empty((B, S, D), np.float32)
    for c in range(8):
        b, hp = c // 4, c % 4
        out[b, 1024 * hp : 1024 * (hp + 1)] = res.results[c]["Y"]
    return out



# revision 2
# speedup vs baseline: 1.3956x; 1.3956x over previous
"""Multi-head causal attention (B=2, S=4096, D=512, H=8, DK=64) on 8 TRN2
NeuronCores — v3: zero-collective row-interleaved sharding.

Core c (b = c//4, hp = c%4) owns batch b's query rows {4l + hp : l in
0..1023} (stride-4 interleave) and computes ALL 8 heads for those rows
end-to-end including the output projection — head mixing needs no
cross-core communication because every core holds complete 512-dim rows.
The full-sequence k/v projections are recomputed on each core of the
batch group (+~55us PE) — cheaper than putting an AllGather on the
critical path, and it removes every collective, every partition-id
conditional, and the output-gather tail of the baseline.

Row interleaving makes the causal workload IDENTICAL on every core: local
q-chunk j (512 cols <-> global rows ~2048j..2048j+2047) needs t-chunks
0..16j+15 on every core; the only per-core divergence is a static [128,32]
boundary mask (data, not code): score (t=128tc+p, q=4(32tc+j')+hp) is
valid iff p <= 4j'+hp.

Everything is bf16 at rest (host pre-casts X and W; rel err 4.2e-3 vs the
2e-2 gate); PSUM accumulation stays f32. Inputs are loaded pre-transposed
straight from DRAM via the HWDGE xbar (dma_start_transpose, 14ns/tile) —
no PE transposes, no staging, no PSUM evacuation for them. Engine budget
(cost model): PE ~186us (attention 113us floor + projections), ScalarE
~147us (exp, sliced to the valid region), DVE ~60us (masks, v interleave,
normalize), Pool ~40us (kT/qT evacuations, partition_broadcast).

Schedule: k/v slice projections are woven INTO the attention t-loops at
their exact dependency steps (slices 2-3 inside chunk-0 head-group 0,
slices 4-7 inside chunk-1 head-group 0 at t=16/20/24/28) so PE never
idles while ScalarE paces the exp-bound stretches; chunk-0's output
projection fills chunk-1's remaining gaps.
"""

import sys

sys.path.insert(0, "/opt/trn_rl_repo")

import numpy as np
import ml_dtypes

import concourse.bass as bass
import concourse.mybir as mybir
import concourse.tile as tile
from concourse import bacc
from concourse.bass import ds, ts
from concourse.bass_utils import run_bass_kernel_spmd
from concourse.masks import make_identity

B, S, D, H, DK = 2, 4096, 512, 8, 64
f32 = mybir.dt.float32
bf16 = mybir.dt.bfloat16
AF = mybir.ActivationFunctionType
BF = ml_dtypes.bfloat16

_CACHED_NC = None


def _attention_chunk(nc, pool, sA, sB, otp, maskT, ident32, kt_full, qt_loc,
                     v_full, ot_all, j, hg, gi0, inject):
    """q-chunk j (local cols 512j..512j+512), head-group hg (heads 2hg,
    2hg+1): t-chunks 0..16j+15, oT accumulated in PSUM across the t loop.
    inject[tc] emits extra (projection) work just before step tc."""
    n_tc = 16 * j + 16
    ot_ps = [None, None]
    pending = None  # software pipeline: oT for step N emits after scores N+1

    def emit_ot(tc, n0, wt):
        for h2 in range(2):
            h = 2 * hg + h2
            if tc == 0:
                ot_ps[h2] = otp.tile([128, 512], f32, tag="otp",
                                     name=f"otp_{j}_{hg}_{h2}")
            nc.tensor.matmul(
                ot_ps[h2][0:65, n0:512],
                lhsT=v_full[:, tc, ds(65 * h, 65)],
                rhs=wt[:, h2, n0:512],
                start=(tc == 0),
                stop=(tc == n_tc - 1),
            )

    for tc in range(n_tc):
        for fn in inject.get(tc, ()):
            fn()
        r = tc - 16 * j
        n0 = 32 * r if r >= 0 else 0
        gi = gi0 + tc
        sp = (sA if gi % 2 == 0 else sB).tile(
            [128, 2, 512], f32, tag="sA" if gi % 2 == 0 else "sB"
        )
        for h2 in range(2):
            if r >= 0:
                # causal mask for the 32-col diagonal block, applied on PE:
                # out[p, j'] = maskT[j', p] (identity-rhs trick); the score
                # matmul then accumulates on top — no cross-engine hop.
                nc.tensor.matmul(
                    sp[:, h2, n0 : n0 + 32], lhsT=maskT, rhs=ident32,
                    start=True, stop=False,
                )
                nc.tensor.matmul(
                    sp[:, h2, n0 : n0 + 32],
                    lhsT=kt_full[64 * h2 : 64 * h2 + 64, hg, ts(tc, 128)],
                    rhs=qt_loc[64 * h2 : 64 * h2 + 64, hg,
                               512 * j + n0 : 512 * j + n0 + 32],
                    start=False,
                    stop=True,
                )
                if n0 + 32 < 512:
                    nc.tensor.matmul(
                        sp[:, h2, n0 + 32 : 512],
                        lhsT=kt_full[64 * h2 : 64 * h2 + 64, hg, ts(tc, 128)],
                        rhs=qt_loc[64 * h2 : 64 * h2 + 64, hg,
                                   512 * j + n0 + 32 : 512 * j + 512],
                        start=True,
                        stop=True,
                    )
            else:
                nc.tensor.matmul(
                    sp[:, h2, 0:512],
                    lhsT=kt_full[64 * h2 : 64 * h2 + 64, hg, ts(tc, 128)],
                    rhs=qt_loc[64 * h2 : 64 * h2 + 64, hg, ts(j, 512)],
                    start=True,
                    stop=True,
                )
        wt = pool("wt", 8).tile([128, 2, 512], bf16, tag="wt")
        nc.scalar.activation(wt[:, :, n0:512], sp[:, :, n0:512], AF.Exp, scale=0.125)
        if pending is not None:
            emit_ot(*pending)
        pending = (tc, n0, wt)
    emit_ot(*pending)
    for h2 in range(2):
        ot_raw = pool("otraw", 4).tile([65, 512], f32, tag="otraw",
                                       name=f"otraw_{j}_{hg}_{h2}")
        nc.vector.tensor_copy(ot_raw, ot_ps[h2][0:65, :])
        recip = pool("recip", 2).tile([1, 512], f32, tag="recip")
        nc.vector.reciprocal(recip, ot_raw[64:65, :])
        rbc = pool("rbc", 2).tile([64, 512], f32, tag="rbc")
        nc.gpsimd.partition_broadcast(rbc, recip)
        nc.vector.tensor_mul(
            ot_all[64 * h2 : 64 * h2 + 64, hg, ts(j, 512)], ot_raw[0:64, :], rbc
        )


def _build_body(nc, tc, Qs, Kf, Vf, Wq, Wk, Wv, Wo, MT, Y):
    ctx_pools = {}

    def pool(name, bufs, space="SBUF"):
        if name not in ctx_pools:
            ctx_pools[name] = tc.alloc_tile_pool(name=name, bufs=bufs, space=space)
        return ctx_pools[name]

    const = pool("const", 1)
    maskT = const.tile([32, 128], bf16, tag="maskT")
    nc.sync.dma_start(maskT, MT[:, :])
    ident32 = const.tile([32, 32], bf16, tag="id32")
    make_identity(nc, ident32)

    kt_full = const.tile([128, 4, 4096], bf16, tag="ktf")
    qt_loc = const.tile([128, 4, 1024], bf16, tag="qtl")
    v_full = const.tile([128, 32, 520], bf16, tag="vf")
    nc.vector.memset(
        v_full.rearrange("p t (h k) -> p t h k", k=65)[:, :, :, 64:65], 1.0
    )
    ot_all = const.tile([128, 4, 1024], bf16, tag="otall")

    mm = pool("mm", 2, space="PSUM")
    sA = pool("sA", 1, space="PSUM")
    sB = pool("sB", 1, space="PSUM")
    otp = pool("otp", 2, space="PSUM")

    # ---- fire transposed input loads (HWDGE xbar); pool depth throttles ----
    xts = {}

    def fire(name, dram, sl):
        t = pool("xt", 14).tile([128, 4, 512], bf16, tag="xt", name=f"xt_{name}")
        nc.sync.dma_start_transpose(t, dram[ds(512 * sl, 512), :])
        xts[name] = t

    # first-needed loads interleaved with their weights on the DMA queue
    fire("k0", Kf, 0)
    wk_sb = const.tile([128, 4, 512], bf16, tag="wk")
    nc.sync.dma_start(wk_sb, Wk.rearrange("(c p) k -> p c k", p=128))
    fire("q0", Qs, 0)
    wq_sb = const.tile([128, 4, 512], bf16, tag="wq")
    nc.sync.dma_start(wq_sb, Wq.rearrange("(c p) k -> p c k", p=128))
    fire("v0", Vf, 0)
    wv_sb = const.tile([128, 4, 512], bf16, tag="wv")
    nc.sync.dma_start(wv_sb, Wv.rearrange("(c p) k -> p c k", p=128))
    fire("k1", Kf, 1)
    fire("v1", Vf, 1)
    fire("q1", Qs, 1)
    wo_sb = const.tile([128, 4, 512], bf16, tag="wo")
    nc.sync.dma_start(wo_sb, Wo.rearrange("(c p) n -> p c n", p=128))
    for sl in range(2, 6):
        fire(f"k{sl}", Kf, sl)
        fire(f"v{sl}", Vf, sl)

    # ---- projection piece emitters (per head-group / t-chunk grain) ----
    def k_piece(sl, hg):
        xt = xts[f"k{sl}"]
        p_ps = mm.tile([128, 512], f32, tag="mm", name=f"pk_{sl}_{hg}")
        for dc in range(4):
            nc.tensor.matmul(
                p_ps,
                lhsT=wk_sb[:, dc, ts(hg, 128)],
                rhs=xt[:, dc, :],
                start=(dc == 0),
                stop=(dc == 3),
            )
        nc.vector.tensor_copy(kt_full[:, hg, ts(sl, 512)], p_ps)

    def v_piece(sl, lc):
        xt = xts[f"v{sl}"]
        vp = mm.tile([128, 512], f32, tag="mm", name=f"pv_{sl}_{lc}")
        for dc in range(4):
            nc.tensor.matmul(
                vp,
                lhsT=xt[:, dc, ts(lc, 128)],
                rhs=wv_sb[:, dc, :],
                start=(dc == 0),
                stop=(dc == 3),
            )
        nc.vector.tensor_copy(
            v_full.rearrange("p t (h k) -> p t h k", k=65)[:, 4 * sl + lc, :, 0:64],
            vp.rearrange("p (h k) -> p h k", h=8),
        )

    def q_piece(sl, hg):
        xt = xts[f"q{sl}"]
        p_ps = mm.tile([128, 512], f32, tag="mm", name=f"pq_{sl}_{hg}")
        for dc in range(4):
            nc.tensor.matmul(
                p_ps,
                lhsT=wq_sb[:, dc, ts(hg, 128)],
                rhs=xt[:, dc, :],
                start=(dc == 0),
                stop=(dc == 3),
            )
        nc.vector.tensor_copy(qt_loc[:, hg, ts(sl, 512)], p_ps)

    def out_proj(j, rbs):
        for rb in rbs:
            ym = mm.tile([128, 512], f32, tag="mm", name=f"ym_{j}_{rb}")
            for dc in range(4):
                nc.tensor.matmul(
                    ym,
                    lhsT=ot_all[:, dc, ds(512 * j + 128 * rb, 128)],
                    rhs=wo_sb[:, dc, :],
                    start=(dc == 0),
                    stop=(dc == 3),
                )
            y_sb = pool("y", 4).tile([128, 512], f32, tag="y")
            if rb % 2 == 0:
                nc.vector.tensor_copy(y_sb, ym)
            else:
                nc.scalar.copy(y_sb, ym)
            nc.sync.dma_start(Y[ts(4 * j + rb, 128), :], y_sb)

    # ---- pre-attention: the minimum for chunk-0 head-group 0 ----
    k_piece(0, 0)
    q_piece(0, 0)
    for lc in range(4):
        v_piece(0, lc)

    # ---- attention, with projection pieces woven at their deadlines ----
    def attn(j, hg, gi0, inject):
        _attention_chunk(nc, pool, sA, sB, otp, maskT, ident32, kt_full,
                         qt_loc, v_full, ot_all, j, hg, gi0, inject)

    def P(*fns):
        return [lambda fns=fns: [f() for f in fns]]

    attn(0, 0, 0, {
        0: P(lambda: fire("k6", Kf, 6), lambda: fire("v6", Vf, 6)),
        4: P(lambda: k_piece(1, 0), lambda: v_piece(1, 0), lambda: v_piece(1, 1)),
        5: P(lambda: v_piece(1, 2), lambda: v_piece(1, 3)),
        8: P(lambda: k_piece(2, 0), lambda: v_piece(2, 0), lambda: v_piece(2, 1)),
        9: P(lambda: v_piece(2, 2), lambda: v_piece(2, 3)),
        12: P(lambda: k_piece(3, 0), lambda: v_piece(3, 0), lambda: v_piece(3, 1)),
        13: P(lambda: v_piece(3, 2), lambda: v_piece(3, 3)),
    })
    for hg in (1, 2, 3):
        attn(0, hg, 16 * hg, {
            0: P(lambda hg=hg: k_piece(0, hg), lambda hg=hg: q_piece(0, hg),
                 *(() if hg != 1 else (lambda: fire("k7", Kf, 7),
                                       lambda: fire("v7", Vf, 7)))),
            4: P(lambda hg=hg: k_piece(1, hg)),
            8: P(lambda hg=hg: k_piece(2, hg)),
            12: P(lambda hg=hg: k_piece(3, hg)),
        })

    attn(1, 0, 64, {
        0: P(lambda: q_piece(1, 0)),
        2: P(lambda: v_piece(4, 0)), 4: P(lambda: v_piece(4, 1)),
        6: P(lambda: v_piece(4, 2)), 8: P(lambda: v_piece(4, 3)),
        10: P(lambda: v_piece(5, 0)), 12: P(lambda: k_piece(4, 0)),
        14: P(lambda: v_piece(5, 1)), 16: P(lambda: v_piece(5, 2)),
        18: P(lambda: v_piece(5, 3), lambda: k_piece(5, 0)),
        21: P(lambda: v_piece(6, 0), lambda: v_piece(6, 1)),
        22: P(lambda: v_piece(6, 2), lambda: v_piece(6, 3), lambda: k_piece(6, 0)),
        26: P(lambda: v_piece(7, 0), lambda: v_piece(7, 1)),
        27: P(lambda: v_piece(7, 2), lambda: v_piece(7, 3), lambda: k_piece(7, 0)),
    })
    for hg in (1, 2, 3):
        attn(1, hg, 64 + 32 * hg, {
            0: P(lambda hg=hg: q_piece(1, hg)),
            8: P() if hg == 3 else P(lambda hg=hg: out_proj(0, (2 * hg - 2, 2 * hg - 1))),
            16: P(lambda hg=hg: k_piece(4, hg)),
            20: P(lambda hg=hg: k_piece(5, hg)),
            24: P(lambda hg=hg: k_piece(6, hg)),
            28: P(lambda hg=hg: k_piece(7, hg)),
        })
    out_proj(1, (0, 1, 2, 3))

    for p in reversed(list(ctx_pools.values())):
        p.release()


def _build():
    global _CACHED_NC
    if _CACHED_NC is not None:
        return _CACHED_NC
    nc = bacc.Bacc("TRN2", num_devices=8)
    Qs = nc.dram_tensor("Qs", [1024, D], bf16, kind="ExternalInput")
    Kf = nc.dram_tensor("Kf", [S, D], bf16, kind="ExternalInput")
    Vf = nc.dram_tensor("Vf", [S, D], bf16, kind="ExternalInput")
    Wq = nc.dram_tensor("Wq", [D, D], bf16, kind="ExternalInput")
    Wk = nc.dram_tensor("Wk", [D, D], bf16, kind="ExternalInput")
    Wv = nc.dram_tensor("Wv", [D, D], bf16, kind="ExternalInput")
    Wo = nc.dram_tensor("Wo", [D, D], bf16, kind="ExternalInput")
    MT = nc.dram_tensor("MT", [32, 128], bf16, kind="ExternalInput")
    Y = nc.dram_tensor("Y", [1024, D], f32, kind="ExternalOutput")
    with tile.TileContext(nc) as tcx:
        _build_body(nc, tcx, Qs, Kf, Vf, Wq, Wk, Wv, Wo, MT, Y)
    nc.finalize()
    _CACHED_NC = nc
    return nc


def _in_maps(inputs):
    Qb = np.asarray(inputs["Q"], np.float32).astype(BF)
    Kb = np.asarray(inputs["K"], np.float32).astype(BF)
    Vb = np.asarray(inputs["V"], np.float32).astype(BF)
    # (h, d, k) -> (d, h*64+k), lhsT layout for the projections
    Wq = np.ascontiguousarray(
        np.asarray(inputs["Wq"], np.float32).transpose(1, 0, 2).reshape(D, D)
    ).astype(BF)
    Wk = np.ascontiguousarray(
        np.asarray(inputs["Wk"], np.float32).transpose(1, 0, 2).reshape(D, D)
    ).astype(BF)
    Wv = np.ascontiguousarray(
        np.asarray(inputs["Wv"], np.float32).transpose(1, 0, 2).reshape(D, D)
    ).astype(BF)
    Wo = np.asarray(inputs["Wo"], np.float32).astype(BF)
    in_maps = []
    for c in range(8):
        b, hp = c // 4, c % 4
        m32 = np.zeros((128, 32), np.float32)
        p = np.arange(128)[:, None]
        jj = np.arange(32)[None, :]
        m32[p > 4 * jj + hp] = -1e10
        mt = np.ascontiguousarray(m32.T).astype(BF)
        in_maps.append(
            {
                "Qs": np.ascontiguousarray(Qb[b, hp::4]),
                "Kf": Kb[b],
                "Vf": Vb[b],
                "Wq": Wq,
                "Wk": Wk,
                "Wv": Wv,
                "Wo": Wo,
                "MT": mt,
            }
        )
    return in_maps


def assemble(per_core_results):
    out = np.empty((B, S, D), np.float32)
    for c in range(8):
        b, hp = c // 4, c % 4
        out[b, hp::4] = per_core_results[c]["Y"]
    return out


def kernel(Q, K, V, Wq, Wk, Wv, Wo):
    nc = _build()
    in_maps = _in_maps(
        {"Q": Q, "K": K, "V": V, "Wq": Wq, "Wk": Wk, "Wv": Wv, "Wo": Wo}
    )
    res = run_bass_kernel_spmd(nc, in_maps, core_ids=list(range(8)))
    return assemble(res.results)
